# revision 5
# baseline (speedup 1.0000x reference)
"""Boundary BCE loss kernel for Trainium2 (8 NeuronCores, data-parallel).

Computes mean(BCEWithLogits(pred, boundary(gt_mask))) where
boundary(m) = 1 iff the 3x3 neighborhood of a pixel (SAME window, valid
elements only) contains both a 0 and a 1.

Layout / algorithm (per core: 8 images of 1024x1024):
  - With *replicate* padding the value-set of a 3x3 window equals the set of
    valid in-bounds values, so boundary <=> 0 < s < 9 where s = replicate-pad
    3x3 count of ones. sum(loss) decomposes as
      sum(softplus(x)) - [sum(x*(s>=0.5)) - sum(x*(s>=8.5))].
  - Each image is row-tiled into 8 conv blocks of 128 input rows starting at
    126k (2-row overlap); block k=0 ("top") emits out rows 0..126 via a
    banded [128,127] bf16 matrix atop (replicate row -1 folded in), blocks
    k>=1 ("int") emit out rows 126k+1..126k+126 via aint whose column 126 is
    ZERO -- the resulting guaranteed s=0 on partition 126 makes the x*(s>=t)
    reductions contribute exactly 0 there, so reduction instructions can run
    on rectangular [127, 2048] tiles spanning a fused PAIR of blocks.
  - FUSION: blocks are processed in pairs (2pi, 2pi+1). One 3D-AP DMA loads
    both gt windows (row stride 126) and one loads both pred windows; the
    mask load is a SWDGE (gpsimd) *casting* DMA int32->bf16, so no engine
    ever spends time casting the mask. The 3x3 conv runs on the tensor
    engine: 3 column-shifted matmuls per 512-col PSUM group (+ tiny N=1
    edge-replicate matmuls reading mf's own edge columns).
  - exp/ln (softplus, Ln's free bias adds the +1) run fused [127,2048] for
    pairs pi>=1: partition 126 of each half then double-counts one row that
    the next block covers again; those sums land isolated in
    acc_sp[126, fused-col] and the HOST subtracts that cell. Pair 0 runs
    exp/ln per-block (exact ranges), since its top half has no spare
    partition. The two x*(s>=t) reductions always run fused (guards give 0).
  - The 8 images' ragged bottom strips (16 in rows / 15 out rows each) are
    stacked into one [128, 1024] block via a 3D DMA and a block-diagonal
    matrix abst, exactly as a normal block.
  - Exp/Ln share one ACT table set (natural_log_exp_and_others; see
    _patch_act_tables) so tables load once.

Each core returns partials [128, 132]; the host sums in float64, subtracts
the fused-pair duplicate cells, and divides by N.
"""

import os
import sys
from collections import deque
from contextlib import ExitStack

import numpy as np

if "/opt/trn_rl_repo" not in sys.path and os.path.isdir("/opt/trn_rl_repo"):
    sys.path.append("/opt/trn_rl_repo")

N_CORES = 8
B, C, H, W = 64, 1, 1024, 1024
IMGS_PER_CORE = B // N_CORES  # 8
P = 128

N_PAIRS = 4          # fused block-pairs per image
N_UNITS = IMGS_PER_CORE * N_PAIRS + 1   # 33 (+1 stacked bottom-strip unit)
SP_COLS = 2 * N_UNITS                   # softplus accum columns (2 per unit)
MB = 15              # bottom strip out rows per image (1024 - (127+7*126))
# units whose exp/ln run fused => host subtracts acc_sp[126, 2*u]
FUSED_UNITS = [N_PAIRS * i + j for i in range(IMGS_PER_CORE) for j in (1, 2, 3)]


def make_consts():
    """Banded vertical-conv matrices A[k, m] = weight of input row k in out m."""
    import ml_dtypes

    bf16 = ml_dtypes.bfloat16

    atop = np.zeros((128, 127), np.float32)
    for m in range(127):
        for k in (m - 1, m, m + 1):
            if 0 <= k < 128:
                atop[k, m] += 1.0
    atop[0, 0] += 1.0  # replicate row -1 -> row 0

    aint = np.zeros((128, 127), np.float32)  # col 126 stays ZERO (guard)
    for m in range(126):
        for k in (m, m + 1, m + 2):
            aint[k, m] += 1.0

    abot = np.zeros((MB + 1, MB), np.float32)
    for m in range(MB):
        for k in (m, m + 1, m + 2):
            if k <= MB:
                abot[k, m] += 1.0
    abot[MB, MB - 1] += 1.0  # replicate row h -> row h-1

    abst = np.zeros((IMGS_PER_CORE * (MB + 1), IMGS_PER_CORE * MB), np.float32)
    for j in range(IMGS_PER_CORE):
        abst[j * (MB + 1) : (j + 1) * (MB + 1), j * MB : (j + 1) * MB] = abot

    return {
        "conv_atop": atop.astype(bf16),
        "conv_aint": aint.astype(bf16),
        "conv_abst": abst.astype(bf16),
    }


def build_program(nc, n_imgs=IMGS_PER_CORE, h=H, w=W):
    """Emit the per-core Tile program onto `nc` (a Bacc)."""
    import concourse.tile as tile
    from concourse import mybir
    from concourse.ap import AP

    f32 = mybir.dt.float32
    i32 = mybir.dt.int32
    bf16 = mybir.dt.bfloat16

    rows = n_imgs * h
    kbs = n_imgs * (MB + 1)   # 128 stacked bottom-strip input rows
    mbs = n_imgs * MB         # 120 stacked bottom-strip output rows

    pred_d = nc.dram_tensor("pred", [rows, w], f32, kind="ExternalInput")
    gt_d = nc.dram_tensor("gt", [rows, w], i32, kind="ExternalInput")
    atop_d = nc.dram_tensor("conv_atop", [128, 127], bf16, kind="ExternalInput")
    aint_d = nc.dram_tensor("conv_aint", [128, 127], bf16, kind="ExternalInput")
    abst_d = nc.dram_tensor("conv_abst", [kbs, mbs], bf16, kind="ExternalInput")
    # partials: [0:SP_COLS) softplus sums, then N_UNITS x*(s>=.5), N_UNITS x*(s>=8.5)
    out_d = nc.dram_tensor("partials", [P, SP_COLS + 2 * N_UNITS], f32,
                           kind="ExternalOutput")

    pred = pred_d.ap()
    gt = gt_d.ap()
    pred3 = pred.rearrange("(j r) c -> j r c", j=n_imgs)
    gt3 = gt.rearrange("(j r) c -> j r c", j=n_imgs)
    out = out_d.ap()

    Exp = mybir.ActivationFunctionType.Exp
    Ln = mybir.ActivationFunctionType.Ln

    with tile.TileContext(nc) as tc, ExitStack() as ctx:
        consts = ctx.enter_context(tc.tile_pool(name="consts", bufs=1))
        xs = ctx.enter_context(tc.tile_pool(name="xs", bufs=8))
        mfs = ctx.enter_context(tc.tile_pool(name="mfs", bufs=6))
        exs = ctx.enter_context(tc.tile_pool(name="exs", bufs=2))
        sps = ctx.enter_context(tc.tile_pool(name="sps", bufs=2))
        ws = ctx.enter_context(tc.tile_pool(name="ws", bufs=4))
        accp = ctx.enter_context(tc.tile_pool(name="accs", bufs=1))
        psum = ctx.enter_context(tc.tile_pool(name="psum", bufs=2, space="PSUM"))

        atop = consts.tile([128, 127], bf16, tag="atop")
        aint = consts.tile([128, 127], bf16, tag="aint")
        abst = consts.tile([kbs, mbs], bf16, tag="abst")
        nc.sync.dma_start(atop[:], atop_d.ap()[:])
        nc.sync.dma_start(aint[:], aint_d.ap()[:])
        nc.sync.dma_start(abst[:], abst_d.ap()[:])

        # one accumulator per producing engine stream
        acc_sp = accp.tile([P, SP_COLS], f32, tag="acc_sp")
        acc_u = accp.tile([P, N_UNITS], f32, tag="acc_u")
        acc_v = accp.tile([P, N_UNITS], f32, tag="acc_v")
        nc.vector.memset(acc_sp[:], 0.0)
        nc.vector.memset(acc_u[:], 0.0)
        nc.vector.memset(acc_v[:], 0.0)

        def conv_half(s2, a, mf2, Bo):
            """3x3 conv of one 1024-col half: 3 shifted matmuls per 512-col
            PSUM group + N=1 edge-replicate matmuls from mf's own edges."""
            mm = nc.tensor.matmul
            mm(s2[:, Bo + 0 : Bo + 512], a[:], mf2[:, Bo + 0 : Bo + 512],
               start=True, stop=False)
            mm(s2[:, Bo + 0 : Bo + 512], a[:], mf2[:, Bo + 1 : Bo + 513],
               start=False, stop=False)
            mm(s2[:, Bo + 1 : Bo + 512], a[:], mf2[:, Bo + 0 : Bo + 511],
               start=False, stop=False)
            mm(s2[:, Bo + 0 : Bo + 1], a[:], mf2[:, Bo + 0 : Bo + 1],
               start=False, stop=True)
            mm(s2[:, Bo + 512 : Bo + 1024], a[:], mf2[:, Bo + 512 : Bo + 1024],
               start=True, stop=False)
            mm(s2[:, Bo + 512 : Bo + 1024], a[:], mf2[:, Bo + 511 : Bo + 1023],
               start=False, stop=False)
            mm(s2[:, Bo + 512 : Bo + 1023], a[:], mf2[:, Bo + 513 : Bo + 1024],
               start=False, stop=False)
            mm(s2[:, Bo + 1023 : Bo + 1024], a[:], mf2[:, Bo + 1023 : Bo + 1024],
               start=False, stop=True)

        def front_pair(img, pi):
            """Loads + conv for fused block pair (2pi, 2pi+1) of one image."""
            in_r0 = img * h + 252 * pi
            mf2 = mfs.tile([128, 2048], bf16, tag="mf")
            nc.gpsimd.dma_start(
                mf2[:],
                AP(gt.tensor, in_r0 * w, [(w, 128), (126 * w, 2), (1, w)]),
            )
            or0 = 0 if pi == 0 else 126 * 2 * pi + 1
            dlt = 127 if pi == 0 else 126
            xr0 = img * h + or0
            # two plain 2D loads: a single 3D-AP HWDGE DMA costs ~4.2us of SP
            # descriptor generation (vs ~0.7us per 2D), throttling the x feed
            x2 = xs.tile([128, 2048], f32, tag="x")
            nc.sync.dma_start(x2[:, 0:1024], pred[xr0 : xr0 + 128, :])
            nc.sync.dma_start(x2[:, 1024:2048],
                              pred[xr0 + dlt : xr0 + dlt + 128, :])
            s2 = psum.tile([127, 2048], f32, tag="s")
            conv_half(s2, atop if pi == 0 else aint, mf2, 0)
            conv_half(s2, aint, mf2, 1024)
            return s2, x2

        def front_bst():
            """Loads + conv for the stacked bottom strips of all 8 images."""
            mfb = mfs.tile([kbs, w], bf16, tag="mf")
            nc.gpsimd.dma_start(mfb[:], gt3[:, h - (MB + 1) : h, :])
            xb = xs.tile([mbs, w], f32, tag="x")
            nc.sync.dma_start(xb[:], pred3[:, h - MB : h, :])
            sb = psum.tile([mbs, w], f32, tag="s")
            conv_half(sb, abst, mfb, 0)
            return sb, xb

        def back_pair(u, fused, s2, x2):
            """Reductions for one fused pair: softplus sums + x*(s>=t) sums."""
            ex2 = exs.tile([127, 2048], f32, tag="ex")
            sp2 = sps.tile([127, 2048], bf16, tag="sp")
            if fused:
                # partition 126 double-counts one row per half; the host
                # subtracts acc_sp[126, 2u] (it contains ONLY those rows)
                nc.scalar.activation(ex2[:], x2[0:127, :], Exp)
                nc.scalar.activation(sp2[:], ex2[:], Ln, bias=1.0,
                                     accum_out=acc_sp[0:127, 2 * u : 2 * u + 1])
            else:
                nc.scalar.activation(ex2[0:127, 0:1024], x2[0:127, 0:1024], Exp)
                nc.scalar.activation(ex2[0:126, 1024:2048], x2[0:126, 1024:2048],
                                     Exp)
                nc.scalar.activation(sp2[0:127, 0:1024], ex2[0:127, 0:1024], Ln,
                                     bias=1.0,
                                     accum_out=acc_sp[0:127, 2 * u : 2 * u + 1])
                nc.scalar.activation(sp2[0:126, 1024:2048], ex2[0:126, 1024:2048],
                                     Ln, bias=1.0,
                                     accum_out=acc_sp[0:126, 2 * u + 1 : 2 * u + 2])
            w1 = ws.tile([127, 2048], bf16, tag="w1")
            nc.vector.scalar_tensor_tensor(
                w1[:], s2[:], 0.5, x2[0:127, :],
                mybir.AluOpType.is_ge, mybir.AluOpType.mult,
                accum_out=acc_u[0:127, u : u + 1],
            )
            w2 = ws.tile([127, 2048], bf16, tag="w2")
            nc.vector.scalar_tensor_tensor(
                w2[:], s2[:], 8.5, x2[0:127, :],
                mybir.AluOpType.is_ge, mybir.AluOpType.mult,
                accum_out=acc_v[0:127, u : u + 1],
            )

        def back_bst(u, sb, xb):
            ex = exs.tile([mbs, w], f32, tag="ex")
            sp = sps.tile([mbs, w], bf16, tag="sp")
            nc.scalar.activation(ex[:], xb[:], Exp)
            nc.scalar.activation(sp[:], ex[:], Ln, bias=1.0,
                                 accum_out=acc_sp[0:mbs, 2 * u : 2 * u + 1])
            w1 = ws.tile([mbs, w], bf16, tag="w1")
            nc.vector.scalar_tensor_tensor(
                w1[:], sb[:], 0.5, xb[:],
                mybir.AluOpType.is_ge, mybir.AluOpType.mult,
                accum_out=acc_u[0:mbs, u : u + 1],
            )
            w2 = ws.tile([mbs, w], bf16, tag="w2")
            nc.vector.scalar_tensor_tensor(
                w2[:], sb[:], 8.5, xb[:],
                mybir.AluOpType.is_ge, mybir.AluOpType.mult,
                accum_out=acc_v[0:mbs, u : u + 1],
            )

        units = [("pair", img, pi) for img in range(n_imgs)
                 for pi in range(N_PAIRS)]
        units.append(("bst",))

        pending = deque()
        for u, spec in enumerate(units):
            if spec[0] == "pair":
                _, img, pi = spec
                front = front_pair(img, pi)
                pending.append(("pair", u, pi != 0, front))
            else:
                front = front_bst()
                pending.append(("bst", u, False, front))
            if len(pending) > 1:
                kind, pu, fused, pf = pending.popleft()
                if kind == "pair":
                    back_pair(pu, fused, *pf)
                else:
                    back_bst(pu, *pf)
        while pending:
            kind, pu, fused, pf = pending.popleft()
            if kind == "pair":
                back_pair(pu, fused, *pf)
            else:
                back_bst(pu, *pf)

        nc.sync.dma_start(out[:, 0:SP_COLS], acc_sp[:])
        nc.sync.dma_start(out[:, SP_COLS : SP_COLS + N_UNITS], acc_u[:])
        nc.sync.dma_start(out[:, SP_COLS + N_UNITS : SP_COLS + 2 * N_UNITS],
                          acc_v[:])


def _patch_act_tables():
    """Make Exp and Ln resolve to the one table set containing both
    (natural_log_exp_and_others); otherwise the table-load pass alternates
    between exp_and_others and natural_log, reloading ~1.3us per activation.
    Set indices (= positions in act_info.json's act_func_sets) are preserved;
    only the membership used for set *selection* is filtered."""
    import concourse.bacc as bacc_mod
    from concourse import mybir

    if getattr(bacc_mod, "_act_tables_patched", False):
        return
    orig = bacc_mod.get_activation_tables
    exp_ln = {mybir.ActivationFunctionType.Exp, mybir.ActivationFunctionType.Ln}

    def patched(arch):
        out = {}
        for name, fns in orig(arch).items():
            out[name] = set(fns) if name == "natural_log_exp_and_others" else (
                set(fns) - exp_ln
            )
        return out

    bacc_mod.get_activation_tables = patched
    bacc_mod._act_tables_patched = True


def _ensure_ntff_hook():
    """Best-effort: make run_bass_kernel_spmd(trace=True) usable. The agent
    container ships no antenv.axon_hooks module, so a BASS_TRACE=1 run would
    otherwise die on the import inside bass_utils. Harmless if unused."""
    try:
        import types

        import antenv

        if "antenv.axon_hooks" in sys.modules:
            return
        m = types.ModuleType("antenv.axon_hooks")
        _h = {}
        m.set_axon_ntff_profile_hook = lambda h: _h.__setitem__("h", h)
        m.get_axon_ntff_profile_hook = lambda: _h.get("h")
        sys.modules["antenv.axon_hooks"] = m
        antenv.axon_hooks = m
        try:
            from trn_agent_boot.trn_boot import _ntff_profile_via_ctypes

            so = "/opt/axon/libaxon_pjrt.so"
            if os.path.exists(so):
                m.set_axon_ntff_profile_hook(_ntff_profile_via_ctypes(so))
        except Exception:
            pass
        try:
            import concourse.bass_utils as bu

            bu.upload_artifacts = lambda tmpdir: tmpdir
        except Exception:
            pass
    except Exception:
        pass


_CACHE = {}


def _get_nc():
    if "nc" not in _CACHE:
        import concourse.bacc as bacc

        _ensure_ntff_hook()
        _patch_act_tables()
        nc = bacc.Bacc("TRN2", target_bir_lowering=False, debug=False,
                       num_devices=N_CORES)
        build_program(nc)
        nc.compile()
        _CACHE["nc"] = nc
    return _CACHE["nc"]


def kernel(pred_boundary: np.ndarray, gt_mask: np.ndarray) -> np.ndarray:
    from concourse.bass_utils import run_bass_kernel_spmd

    nc = _get_nc()
    consts = make_consts()

    pred = np.ascontiguousarray(pred_boundary, dtype=np.float32).reshape(B * H, W)
    gt = np.ascontiguousarray(gt_mask, dtype=np.int32).reshape(B * H, W)

    rows_per_core = IMGS_PER_CORE * H
    in_maps = []
    for c in range(N_CORES):
        r0 = c * rows_per_core
        in_maps.append(
            {
                "pred": pred[r0 : r0 + rows_per_core],
                "gt": gt[r0 : r0 + rows_per_core],
                **consts,
            }
        )

    res = run_bass_kernel_spmd(nc, in_maps, list(range(N_CORES)))
    _CACHE["last_results"] = res

    fused_sp_cols = [2 * u for u in FUSED_UNITS]
    total = np.float64(0.0)
    for c in range(N_CORES):
        p = res.results[c]["partials"].astype(np.float64)
        sp = p[:, 0:SP_COLS].sum() - p[126, fused_sp_cols].sum()
        xu = p[:, SP_COLS : SP_COLS + N_UNITS].sum()
        xv = p[:, SP_COLS + N_UNITS : SP_COLS + 2 * N_UNITS].sum()
        total += sp - (xu - xv)

    mean = total / float(B * C * H * W)
    return np.float32(mean)


# revision 6
# speedup vs baseline: 1.0398x; 1.0398x over previous
"""Boundary BCE loss kernel for Trainium2 (8 NeuronCores, data-parallel).

Computes mean(BCEWithLogits(pred, boundary(gt_mask))) where
boundary(m) = 1 iff the 3x3 neighborhood of a pixel (SAME window, valid
elements only) contains both a 0 and a 1.

Layout / algorithm (per core: 8 images of 1024x1024):
  - With *replicate* padding the value-set of a 3x3 window equals the set of
    valid in-bounds values, so boundary <=> 0 < s < 9 where s = replicate-pad
    3x3 count of ones. sum(loss) decomposes as
      sum(softplus(x)) - [sum(x*(s>=0.5)) - sum(x*(s>=8.5))].
  - Each image is row-tiled into 8 conv blocks of 128 input rows starting at
    126k (2-row overlap); block k=0 ("top") emits out rows 0..126 via a
    banded [128,127] bf16 matrix atop (replicate row -1 folded in), blocks
    k>=1 ("int") emit out rows 126k+1..126k+126 via aint whose column 126 is
    ZERO -- the resulting guaranteed s=0 on partition 126 makes the x*(s>=t)
    reductions contribute exactly 0 there, so reduction instructions can run
    on rectangular [127, 2048] tiles spanning a fused PAIR of blocks.
  - FUSION: blocks are processed in pairs (2pi, 2pi+1). One 3D-AP DMA loads
    both gt windows (row stride 126) and one loads both pred windows; the
    mask load is a SWDGE (gpsimd) *casting* DMA int32->bf16, so no engine
    ever spends time casting the mask. The 3x3 conv runs on the tensor
    engine: 3 column-shifted matmuls per 512-col PSUM group (+ tiny N=1
    edge-replicate matmuls reading mf's own edge columns).
  - exp/ln (softplus, Ln's free bias adds the +1) run fused [127,2048] for
    pairs pi>=1: partition 126 of each half then double-counts one row that
    the next block covers again; those sums land isolated in
    acc_sp[126, fused-col] and the HOST subtracts that cell. Pair 0 runs
    exp/ln per-block (exact ranges), since its top half has no spare
    partition. The two x*(s>=t) reductions always run fused (guards give 0).
  - The 8 images' ragged bottom strips (16 in rows / 15 out rows each) are
    stacked into one [128, 1024] block via a 3D DMA and a block-diagonal
    matrix abst, exactly as a normal block.
  - Exp/Ln share one ACT table set (natural_log_exp_and_others; see
    _patch_act_tables) so tables load once.

Each core returns partials [128, 132]; the host sums in float64, subtracts
the fused-pair duplicate cells, and divides by N.
"""

import os
import sys
from collections import deque
from contextlib import ExitStack

import numpy as np

if "/opt/trn_rl_repo" not in sys.path and os.path.isdir("/opt/trn_rl_repo"):
    sys.path.append("/opt/trn_rl_repo")

N_CORES = 8
B, C, H, W = 64, 1, 1024, 1024
IMGS_PER_CORE = B // N_CORES  # 8
P = 128

N_PAIRS = 4          # fused block-pairs per image
N_UNITS = IMGS_PER_CORE * N_PAIRS + 1   # 33 (+1 stacked bottom-strip unit)
SP_COLS = 2 * N_UNITS                   # softplus accum columns (2 per unit)
MB = 15              # bottom strip out rows per image (1024 - (127+7*126))
# units whose exp/ln run fused => host subtracts acc_sp[126, 2*u]
FUSED_UNITS = [N_PAIRS * i + j for i in range(IMGS_PER_CORE) for j in (1, 2, 3)]


def make_consts():
    """Banded vertical-conv matrices A[k, m] = weight of input row k in out m."""
    import ml_dtypes

    bf16 = ml_dtypes.bfloat16

    atop = np.zeros((128, 127), np.float32)
    for m in range(127):
        for k in (m - 1, m, m + 1):
            if 0 <= k < 128:
                atop[k, m] += 1.0
    atop[0, 0] += 1.0  # replicate row -1 -> row 0

    aint = np.zeros((128, 127), np.float32)  # col 126 stays ZERO (guard)
    for m in range(126):
        for k in (m, m + 1, m + 2):
            aint[k, m] += 1.0

    abot = np.zeros((MB + 1, MB), np.float32)
    for m in range(MB):
        for k in (m, m + 1, m + 2):
            if k <= MB:
                abot[k, m] += 1.0
    abot[MB, MB - 1] += 1.0  # replicate row h -> row h-1

    abst = np.zeros((IMGS_PER_CORE * (MB + 1), IMGS_PER_CORE * MB), np.float32)
    for j in range(IMGS_PER_CORE):
        abst[j * (MB + 1) : (j + 1) * (MB + 1), j * MB : (j + 1) * MB] = abot

    return {
        "conv_atop": atop.astype(bf16),
        "conv_aint": aint.astype(bf16),
        "conv_abst": abst.astype(bf16),
    }


def build_program(nc, n_imgs=IMGS_PER_CORE, h=H, w=W):
    """Emit the per-core Tile program onto `nc` (a Bacc)."""
    import concourse.tile as tile
    from concourse import mybir
    from concourse.ap import AP

    f32 = mybir.dt.float32
    i32 = mybir.dt.int32
    bf16 = mybir.dt.bfloat16

    rows = n_imgs * h
    kbs = n_imgs * (MB + 1)   # 128 stacked bottom-strip input rows
    mbs = n_imgs * MB         # 120 stacked bottom-strip output rows

    pred_d = nc.dram_tensor("pred", [rows, w], f32, kind="ExternalInput")
    gt_d = nc.dram_tensor("gt", [rows, w], i32, kind="ExternalInput")
    atop_d = nc.dram_tensor("conv_atop", [128, 127], bf16, kind="ExternalInput")
    aint_d = nc.dram_tensor("conv_aint", [128, 127], bf16, kind="ExternalInput")
    abst_d = nc.dram_tensor("conv_abst", [kbs, mbs], bf16, kind="ExternalInput")
    # partials: [0:SP_COLS) softplus sums, then N_UNITS x*(s>=.5), N_UNITS x*(s>=8.5)
    out_d = nc.dram_tensor("partials", [P, SP_COLS + 2 * N_UNITS], f32,
                           kind="ExternalOutput")

    pred = pred_d.ap()
    gt = gt_d.ap()
    pred3 = pred.rearrange("(j r) c -> j r c", j=n_imgs)
    gt3 = gt.rearrange("(j r) c -> j r c", j=n_imgs)
    out = out_d.ap()

    Exp = mybir.ActivationFunctionType.Exp
    Ln = mybir.ActivationFunctionType.Ln

    with tile.TileContext(nc) as tc, ExitStack() as ctx:
        consts = ctx.enter_context(tc.tile_pool(name="consts", bufs=1))
        xs = ctx.enter_context(tc.tile_pool(name="xs", bufs=8))
        mfs = ctx.enter_context(tc.tile_pool(name="mfs", bufs=6))
        exs = ctx.enter_context(tc.tile_pool(name="exs", bufs=2))
        sps = ctx.enter_context(tc.tile_pool(name="sps", bufs=2))
        ws = ctx.enter_context(tc.tile_pool(name="ws", bufs=4))
        accp = ctx.enter_context(tc.tile_pool(name="accs", bufs=1))
        psum = ctx.enter_context(tc.tile_pool(name="psum", bufs=2, space="PSUM"))

        atop = consts.tile([128, 127], bf16, tag="atop")
        aint = consts.tile([128, 127], bf16, tag="aint")
        abst = consts.tile([kbs, mbs], bf16, tag="abst")
        nc.sync.dma_start(atop[:], atop_d.ap()[:])
        nc.sync.dma_start(aint[:], aint_d.ap()[:])
        nc.sync.dma_start(abst[:], abst_d.ap()[:])

        # one accumulator per producing engine stream
        acc_sp = accp.tile([P, SP_COLS], f32, tag="acc_sp")
        acc_u = accp.tile([P, N_UNITS], f32, tag="acc_u")
        acc_v = accp.tile([P, N_UNITS], f32, tag="acc_v")
        nc.vector.memset(acc_sp[:], 0.0)
        nc.vector.memset(acc_u[:], 0.0)
        nc.vector.memset(acc_v[:], 0.0)

        def conv_half(s2, a, mf2, Bo):
            """3x3 conv of one 1024-col half: 3 shifted matmuls per 512-col
            PSUM group + N=1 edge-replicate matmuls from mf's own edges."""
            mm = nc.tensor.matmul
            mm(s2[:, Bo + 0 : Bo + 512], a[:], mf2[:, Bo + 0 : Bo + 512],
               start=True, stop=False)
            mm(s2[:, Bo + 0 : Bo + 512], a[:], mf2[:, Bo + 1 : Bo + 513],
               start=False, stop=False)
            mm(s2[:, Bo + 1 : Bo + 512], a[:], mf2[:, Bo + 0 : Bo + 511],
               start=False, stop=False)
            mm(s2[:, Bo + 0 : Bo + 1], a[:], mf2[:, Bo + 0 : Bo + 1],
               start=False, stop=True)
            mm(s2[:, Bo + 512 : Bo + 1024], a[:], mf2[:, Bo + 512 : Bo + 1024],
               start=True, stop=False)
            mm(s2[:, Bo + 512 : Bo + 1024], a[:], mf2[:, Bo + 511 : Bo + 1023],
               start=False, stop=False)
            mm(s2[:, Bo + 512 : Bo + 1023], a[:], mf2[:, Bo + 513 : Bo + 1024],
               start=False, stop=False)
            mm(s2[:, Bo + 1023 : Bo + 1024], a[:], mf2[:, Bo + 1023 : Bo + 1024],
               start=False, stop=True)

        def front_pair(img, pi):
            """Loads + conv for fused block pair (2pi, 2pi+1) of one image."""
            in_r0 = img * h + 252 * pi
            mf2 = mfs.tile([128, 2048], bf16, tag="mf")
            nc.gpsimd.dma_start(
                mf2[:],
                AP(gt.tensor, in_r0 * w, [(w, 128), (126 * w, 2), (1, w)]),
            )
            or0 = 0 if pi == 0 else 126 * 2 * pi + 1
            dlt = 127 if pi == 0 else 126
            xr0 = img * h + or0
            # two plain 2D loads: a single 3D-AP HWDGE DMA costs ~4.2us of SP
            # descriptor generation (vs ~0.7us per 2D), throttling the x feed
            x2 = xs.tile([128, 2048], f32, tag="x")
            nc.sync.dma_start(x2[:, 0:1024], pred[xr0 : xr0 + 128, :])
            nc.sync.dma_start(x2[:, 1024:2048],
                              pred[xr0 + dlt : xr0 + dlt + 128, :])
            s2 = psum.tile([127, 2048], f32, tag="s")
            conv_half(s2, atop if pi == 0 else aint, mf2, 0)
            conv_half(s2, aint, mf2, 1024)
            return s2, x2

        def front_bst():
            """Loads + conv for the stacked bottom strips of all 8 images."""
            mfb = mfs.tile([kbs, w], bf16, tag="mf")
            nc.gpsimd.dma_start(mfb[:], gt3[:, h - (MB + 1) : h, :])
            xb = xs.tile([mbs, w], f32, tag="x")
            nc.sync.dma_start(xb[:], pred3[:, h - MB : h, :])
            sb = psum.tile([mbs, w], f32, tag="s")
            conv_half(sb, abst, mfb, 0)
            return sb, xb

        def back_pair(u, fused, s2, x2):
            """Reductions for one fused pair: softplus sums + x*(s>=t) sums."""
            ex2 = exs.tile([127, 2048], f32, tag="ex")
            sp2 = sps.tile([127, 2048], bf16, tag="sp")
            if fused:
                # partition 126 double-counts one row per half; the host
                # subtracts acc_sp[126, 2u] (it contains ONLY those rows)
                nc.scalar.activation(ex2[:], x2[0:127, :], Exp)
                nc.scalar.activation(sp2[:], ex2[:], Ln, bias=1.0,
                                     accum_out=acc_sp[0:127, 2 * u : 2 * u + 1])
            else:
                nc.scalar.activation(ex2[0:127, 0:1024], x2[0:127, 0:1024], Exp)
                nc.scalar.activation(ex2[0:126, 1024:2048], x2[0:126, 1024:2048],
                                     Exp)
                nc.scalar.activation(sp2[0:127, 0:1024], ex2[0:127, 0:1024], Ln,
                                     bias=1.0,
                                     accum_out=acc_sp[0:127, 2 * u : 2 * u + 1])
                nc.scalar.activation(sp2[0:126, 1024:2048], ex2[0:126, 1024:2048],
                                     Ln, bias=1.0,
                                     accum_out=acc_sp[0:126, 2 * u + 1 : 2 * u + 2])
            w1 = ws.tile([127, 2048], bf16, tag="w1")
            nc.vector.scalar_tensor_tensor(
                w1[:], s2[:], 0.5, x2[0:127, :],
                mybir.AluOpType.is_ge, mybir.AluOpType.mult,
                accum_out=acc_u[0:127, u : u + 1],
            )
            w2 = ws.tile([127, 2048], bf16, tag="w2")
            nc.vector.scalar_tensor_tensor(
                w2[:], s2[:], 8.5, x2[0:127, :],
                mybir.AluOpType.is_ge, mybir.AluOpType.mult,
                accum_out=acc_v[0:127, u : u + 1],
            )

        def back_bst(u, sb, xb):
            ex = exs.tile([mbs, w], f32, tag="ex")
            sp = sps.tile([mbs, w], bf16, tag="sp")
            nc.scalar.activation(ex[:], xb[:], Exp)
            nc.scalar.activation(sp[:], ex[:], Ln, bias=1.0,
                                 accum_out=acc_sp[0:mbs, 2 * u : 2 * u + 1])
            w1 = ws.tile([mbs, w], bf16, tag="w1")
            nc.vector.scalar_tensor_tensor(
                w1[:], sb[:], 0.5, xb[:],
                mybir.AluOpType.is_ge, mybir.AluOpType.mult,
                accum_out=acc_u[0:mbs, u : u + 1],
            )
            w2 = ws.tile([mbs, w], bf16, tag="w2")
            nc.vector.scalar_tensor_tensor(
                w2[:], sb[:], 8.5, xb[:],
                mybir.AluOpType.is_ge, mybir.AluOpType.mult,
                accum_out=acc_v[0:mbs, u : u + 1],
            )

        units = [("pair", img, pi) for img in range(n_imgs)
                 for pi in range(N_PAIRS)]
        units.append(("bst",))

        pending = deque()
        for u, spec in enumerate(units):
            if spec[0] == "pair":
                _, img, pi = spec
                front = front_pair(img, pi)
                pending.append(("pair", u, pi != 0, front))
            else:
                front = front_bst()
                pending.append(("bst", u, False, front))
            if len(pending) > 2:
                kind, pu, fused, pf = pending.popleft()
                if kind == "pair":
                    back_pair(pu, fused, *pf)
                else:
                    back_bst(pu, *pf)
        while pending:
            kind, pu, fused, pf = pending.popleft()
            if kind == "pair":
                back_pair(pu, fused, *pf)
            else:
                back_bst(pu, *pf)

        nc.sync.dma_start(out[:, 0:SP_COLS], acc_sp[:])
        nc.sync.dma_start(out[:, SP_COLS : SP_COLS + N_UNITS], acc_u[:])
        nc.sync.dma_start(out[:, SP_COLS + N_UNITS : SP_COLS + 2 * N_UNITS],
                          acc_v[:])


def _patch_act_tables():
    """Make Exp and Ln resolve to the one table set containing both
    (natural_log_exp_and_others); otherwise the table-load pass alternates
    between exp_and_others and natural_log, reloading ~1.3us per activation.
    Set indices (= positions in act_info.json's act_func_sets) are preserved;
    only the membership used for set *selection* is filtered."""
    import concourse.bacc as bacc_mod
    from concourse import mybir

    if getattr(bacc_mod, "_act_tables_patched", False):
        return
    orig = bacc_mod.get_activation_tables
    exp_ln = {mybir.ActivationFunctionType.Exp, mybir.ActivationFunctionType.Ln}

    def patched(arch):
        out = {}
        for name, fns in orig(arch).items():
            out[name] = set(fns) if name == "natural_log_exp_and_others" else (
                set(fns) - exp_ln
            )
        return out

    bacc_mod.get_activation_tables = patched
    bacc_mod._act_tables_patched = True


def _ensure_ntff_hook():
    """Best-effort: make run_bass_kernel_spmd(trace=True) usable. The agent
    container ships no antenv.axon_hooks module, so a BASS_TRACE=1 run would
    otherwise die on the import inside bass_utils. Harmless if unused."""
    try:
        import types

        import antenv

        if "antenv.axon_hooks" in sys.modules:
            return
        m = types.ModuleType("antenv.axon_hooks")
        _h = {}
        m.set_axon_ntff_profile_hook = lambda h: _h.__setitem__("h", h)
        m.get_axon_ntff_profile_hook = lambda: _h.get("h")
        sys.modules["antenv.axon_hooks"] = m
        antenv.axon_hooks = m
        try:
            from trn_agent_boot.trn_boot import _ntff_profile_via_ctypes

            so = "/opt/axon/libaxon_pjrt.so"
            if os.path.exists(so):
                m.set_axon_ntff_profile_hook(_ntff_profile_via_ctypes(so))
        except Exception:
            pass
        try:
            import concourse.bass_utils as bu

            bu.upload_artifacts = lambda tmpdir: tmpdir
        except Exception:
            pass
    except Exception:
        pass


_CACHE = {}


def _get_nc():
    if "nc" not in _CACHE:
        import concourse.bacc as bacc

        _ensure_ntff_hook()
        _patch_act_tables()
        nc = bacc.Bacc("TRN2", target_bir_lowering=False, debug=False,
                       num_devices=N_CORES)
        build_program(nc)
        nc.compile()
        _CACHE["nc"] = nc
    return _CACHE["nc"]


def kernel(pred_boundary: np.ndarray, gt_mask: np.ndarray) -> np.ndarray:
    from concourse.bass_utils import run_bass_kernel_spmd

    nc = _get_nc()
    consts = make_consts()

    pred = np.ascontiguousarray(pred_boundary, dtype=np.float32).reshape(B * H, W)
    gt = np.ascontiguousarray(gt_mask, dtype=np.int32).reshape(B * H, W)

    rows_per_core = IMGS_PER_CORE * H
    in_maps = []
    for c in range(N_CORES):
        r0 = c * rows_per_core
        in_maps.append(
            {
                "pred": pred[r0 : r0 + rows_per_core],
                "gt": gt[r0 : r0 + rows_per_core],
                **consts,
            }
        )

    res = run_bass_kernel_spmd(nc, in_maps, list(range(N_CORES)))
    _CACHE["last_results"] = res

    fused_sp_cols = [2 * u for u in FUSED_UNITS]
    total = np.float64(0.0)
    for c in range(N_CORES):
        p = res.results[c]["partials"].astype(np.float64)
        sp = p[:, 0:SP_COLS].sum() - p[126, fused_sp_cols].sum()
        xu = p[:, SP_COLS : SP_COLS + N_UNITS].sum()
        xv = p[:, SP_COLS + N_UNITS : SP_COLS + 2 * N_UNITS].sum()
        total += sp - (xu - xv)

    mean = total / float(B * C * H * W)
    return np.float32(mean)
